# revision 13
# baseline (speedup 1.0000x reference)
"""Trainium2 Bass kernel for nn_MetaSpace (keypoint gaussian pooling + meta-space fusion).

Strategy (sharding): data-parallel over batch B: 4 batches per core x 8 cores.
meta / all linear weights replicated.

Gather: each core streams its 4 feature maps channel-half by channel-half into
SBUF with dense, descriptor-optimal HWDGE DMAs, then extracts the 5x5 patch
around each keypoint with register-offset dynamic access patterns on the DVE
(keypoint coords are data -> loaded into engine registers from an int32 input).
Gaussian pooling, the tiny MLP branch and the 2-token gated MHA (softmax over 2
keys done in closed form via sigmoid) all run on-chip in fp32.

Pipeline per core (68 tokens = 4 batches x 17 keypoints):
  dense fm load -> dynamic-AP 5x5 extract -> gaussian pool -> kpt_feat^T [2x128, 68]
  -> PE matmuls vs Wq/Wk/Wv/Wg/P1w (token-major [68,256] PSUM outputs)
  -> closed-form 2-token softmax + headwise gating + token mean
  -> LayerNorm+ReLU branch -> PE transposes -> final Wo/P2w matmuls in one PSUM
  -> out [68, 256] -> DRAM.
"""

import os

import numpy as np
from contextlib import ExitStack

import concourse.bass as bass
import concourse.bacc as bacc
import concourse.tile as tile
import concourse.mybir as mybir
from concourse.bass_utils import run_bass_kernel_spmd
from concourse.masks import make_identity

# ---- problem constants (hardcoded; kernel.py must be self-contained) ----
B, N, C, H, W = 32, 17, 256, 96, 72
NCORES = 8
BPC = B // NCORES          # batches per core = 4
T = BPC * N                # tokens per core = 68
NH = 8                     # heads
DK = C // NH               # 32
KSZ = 5
HALF = 2
SIGMA = 2.0
ORIG_H, ORIG_W = 384, 288
EPS = 1e-5
PK = KSZ * KSZ             # 25
PATCH_F = 2 * T * PK       # patches free size = 3400

F32 = mybir.dt.float32
I32 = mybir.dt.int32
AF = mybir.ActivationFunctionType
ALU = mybir.AluOpType
AX = mybir.AxisListType

_NC_CACHE = {}


def _build_program():
    nc = bacc.Bacc()

    # DRAM I/O
    fm = nc.declare_dram_parameter("fm", [BPC * C, H, W], F32, isOutput=False)
    yoff = nc.declare_dram_parameter("yoff", [1, T], I32, isOutput=False)
    xoff = nc.declare_dram_parameter("xoff", [1, T], I32, isOutput=False)
    gker = nc.declare_dram_parameter("gker", [128, N * PK], F32, isOutput=False)
    metat = nc.declare_dram_parameter("metat", [128, 2 * T], F32, isOutput=False)
    wq = nc.declare_dram_parameter("wq", [C, C], F32, isOutput=False)
    wk = nc.declare_dram_parameter("wk", [C, C], F32, isOutput=False)
    wv = nc.declare_dram_parameter("wv", [C, C], F32, isOutput=False)
    wo = nc.declare_dram_parameter("wo", [C, C], F32, isOutput=False)
    p1w = nc.declare_dram_parameter("p1w", [2 * C, C], F32, isOutput=False)
    p2w = nc.declare_dram_parameter("p2w", [C, C], F32, isOutput=False)
    wg = nc.declare_dram_parameter("wg", [C, NH], F32, isOutput=False)
    ehalf = nc.declare_dram_parameter("ehalf", [NH, C], F32, isOutput=False)
    bq_r = nc.declare_dram_parameter("bq_r", [T, C], F32, isOutput=False)
    bv_r = nc.declare_dram_parameter("bv_r", [T, C], F32, isOutput=False)
    p1b_r = nc.declare_dram_parameter("p1b_r", [T, C], F32, isOutput=False)
    lng_r = nc.declare_dram_parameter("lng_r", [T, C], F32, isOutput=False)
    lnb_r = nc.declare_dram_parameter("lnb_r", [T, C], F32, isOutput=False)
    bfin_r = nc.declare_dram_parameter("bfin_r", [T, C], F32, isOutput=False)
    bg_r = nc.declare_dram_parameter("bg_r", [T, NH], F32, isOutput=False)
    out = nc.declare_dram_parameter("out", [T, C], F32, isOutput=True)
    dbg = os.environ.get("K_DEBUG") == "1"
    if dbg:
        d_xt = nc.declare_dram_parameter("d_xt", [128, 2 * T], F32, isOutput=True)
        d_p0 = nc.declare_dram_parameter("d_p0", [128, PATCH_F], F32, isOutput=True)

    with ExitStack() as ctx:
        tc = ctx.enter_context(tile.TileContext(nc))
        const = ctx.enter_context(tc.tile_pool(name="const", bufs=1))
        work = ctx.enter_context(tc.tile_pool(name="work", bufs=1))
        fmpool = ctx.enter_context(tc.tile_pool(name="fmpool", bufs=2))
        # PSUM pools: [68,256] matmul outs rotate in 3 banks; gates need 2 live;
        # transposes rotate in 2; [8,68] small transposes in 1.  3+2+2+1 = 8 banks.
        ps_mm = ctx.enter_context(tc.tile_pool(name="ps_mm", bufs=3, space="PSUM"))
        ps_g = ctx.enter_context(tc.tile_pool(name="ps_g", bufs=2, space="PSUM"))
        ps_tp = ctx.enter_context(tc.tile_pool(name="ps_tp", bufs=2, space="PSUM"))
        ps_sm = ctx.enter_context(tc.tile_pool(name="ps_sm", bufs=1, space="PSUM"))

        # ---- stage constants / weights (HWDGE), issue early so they overlap fm loads
        yx_t = const.tile([1, 2 * T], I32, tag="yx")
        nc.sync.dma_start(yx_t[:, :T], yoff[:])
        nc.sync.dma_start(yx_t[:, T:], xoff[:])
        g_t = const.tile([128, N * PK], F32, tag="gker")
        nc.sync.dma_start(g_t[:], gker[:])
        mt_t = const.tile([128, 2 * T], F32, tag="metat")
        nc.sync.dma_start(mt_t[:], metat[:])

        def load_slices(dram, rows, cols, tagbase):
            tiles = []
            for k in range(rows // 128):
                t_ = const.tile([128, cols], F32, tag=f"{tagbase}{k}")
                nc.sync.dma_start(t_[:], dram[128 * k : 128 * (k + 1), :])
                tiles.append(t_)
            return tiles

        wq_s = load_slices(wq, C, C, "wq")
        wk_s = load_slices(wk, C, C, "wk")
        wv_s = load_slices(wv, C, C, "wv")
        wo_s = load_slices(wo, C, C, "wo")
        p1w_s = load_slices(p1w, 2 * C, C, "p1w")
        p2w_s = load_slices(p2w, C, C, "p2w")
        wg_s = load_slices(wg, C, NH, "wg")

        eh_t = const.tile([NH, C], F32, tag="ehalf")
        nc.sync.dma_start(eh_t[:], ehalf[:])

        def load_rep(dram, cols, tag):
            t_ = const.tile([T, cols], F32, tag=tag)
            nc.sync.dma_start(t_[:], dram[:])
            return t_

        bq_t = load_rep(bq_r, C, "bq")
        bv_t = load_rep(bv_r, C, "bv")
        p1b_t = load_rep(p1b_r, C, "p1b")
        lng_t = load_rep(lng_r, C, "lng")
        lnb_t = load_rep(lnb_r, C, "lnb")
        bfin_t = load_rep(bfin_r, C, "bfin")
        bg_t = load_rep(bg_r, NH, "bg")

        id_t = const.tile([128, 128], F32, tag="ident")
        make_identity(nc, id_t[:])
        id68 = id_t[:T, :T]

        # ---- dense fm stream + dynamic-AP 5x5 extraction + gaussian pooling
        # patches[:, ct*T + t, :] = fm[bl, ctile_c, y_t-2:y_t+3, x_t-2:x_t+3]
        patches = const.tile([128, PATCH_F], F32, tag="patches")
        xt = const.tile([128, 2 * T], F32, tag="xt")
        for bl in range(BPC):
            fmt = []
            for ct in range(2):
                ft = fmpool.tile([128, H * W], F32, tag=f"fm{ct}")
                nc.sync.dma_start(
                    ft[:],
                    fm[bl * C + ct * 128 : bl * C + ct * 128 + 128, :, :].rearrange(
                        "c y x -> c (y x)"
                    ),
                )
                fmt.append(ft)
            for n in range(N):
                t = bl * N + n
                ry = nc.vector.alloc_register(f"ry{t}")
                nc.vector.reg_load(ry, yx_t[0:1, t : t + 1])
                sy = nc.vector.snap(ry, donate=True, min_val=0, max_val=H - KSZ)
                rx = nc.vector.alloc_register(f"rx{t}")
                nc.vector.reg_load(rx, yx_t[0:1, T + t : T + t + 1])
                sx = nc.vector.snap(rx, donate=True, min_val=0, max_val=W - KSZ)
                for ct in range(2):
                    view = fmt[ct][:].rearrange("p (y x) -> p y x", x=W)
                    dst = patches[
                        :, (ct * T + t) * PK : (ct * T + t + 1) * PK
                    ].rearrange("p (a b) -> p a b", b=KSZ)
                    nc.vector.tensor_copy(
                        dst, view[:, bass.ds(sy, KSZ), bass.ds(sx, KSZ)]
                    )
            # gaussian pooling for this batch, both channel halves
            for ct in range(2):
                csl = slice((ct * T + bl * N) * PK, (ct * T + bl * N + N) * PK)
                wt = work.tile([128, N * PK], F32, tag="wtmp")
                nc.vector.tensor_tensor(
                    out=wt[:], in0=patches[:, csl], in1=g_t[:], op=ALU.mult
                )
                nc.vector.reduce_sum(
                    out=xt[:, ct * T + bl * N : ct * T + bl * N + N],
                    in_=wt[:].rearrange("p (k r) -> p k r", r=PK),
                    axis=AX.X,
                )

        if dbg:
            nc.sync.dma_start(d_xt[:], xt[:])
            nc.sync.dma_start(d_p0[:], patches[:])

        # lhsT slices: kpt tokens (xt) and meta tokens (mt_t), per channel-half
        xk = [xt[:, :T], xt[:, T:]]
        xm = [mt_t[:, :T], mt_t[:, T:]]

        def mm2(lhs_pair, rhs_pair, pool, n):
            ps = pool.tile([T, n], F32, space="PSUM", tag="mm")
            nc.tensor.matmul(ps[:], lhs_pair[0], rhs_pair[0][:], start=True, stop=False)
            nc.tensor.matmul(ps[:], lhs_pair[1], rhs_pair[1][:], start=False, stop=True)
            return ps

        # negated meta activations let PE accumulate K/V *differences* in PSUM
        # (walrus forbids reading two PSUM operands in one DVE op)
        xmn_t = const.tile([128, 2 * T], F32, tag="xmn")
        nc.scalar.mul(xmn_t[:], mt_t[:], -1.0)
        xmn = [xmn_t[:, :T], xmn_t[:, T:]]

        def mm4(lhs_a, lhs_b, rhs_pair, pool, n):
            ps = pool.tile([T, n], F32, space="PSUM", tag="mm")
            nc.tensor.matmul(ps[:], lhs_a[0], rhs_pair[0][:], start=True, stop=False)
            nc.tensor.matmul(ps[:], lhs_a[1], rhs_pair[1][:], start=False, stop=False)
            nc.tensor.matmul(ps[:], lhs_b[0], rhs_pair[0][:], start=False, stop=False)
            nc.tensor.matmul(ps[:], lhs_b[1], rhs_pair[1][:], start=False, stop=True)
            return ps

        # ---- K/V first so their PSUM banks recycle fastest
        dk_ps = mm4(xk, xmn, wk_s, ps_mm, C)  # K_kpt - K_meta, stays in PSUM

        dv_ps = mm4(xk, xmn, wv_s, ps_mm, C)  # V_kpt - V_meta
        dv_t = work.tile([T, C], F32, tag="dv")
        nc.vector.tensor_copy(dv_t[:], dv_ps[:])
        vm_ps = mm2(xm, wv_s, ps_mm, C)
        vm_t = work.tile([T, C], F32, tag="vm")
        nc.vector.tensor_tensor(out=vm_t[:], in0=vm_ps[:], in1=bv_t[:], op=ALU.add)

        qk_ps = mm2(xk, wq_s, ps_mm, C)
        qk_t = work.tile([T, C], F32, tag="qk")
        nc.vector.tensor_tensor(out=qk_t[:], in0=qk_ps[:], in1=bq_t[:], op=ALU.add)
        qm_ps = mm2(xm, wq_s, ps_mm, C)
        qm_t = work.tile([T, C], F32, tag="qm")
        nc.vector.tensor_tensor(out=qm_t[:], in0=qm_ps[:], in1=bq_t[:], op=ALU.add)

        # ---- gates: sigmoid(x @ Wg + bg)  [68, 8]
        gk_ps = mm2(xk, wg_s, ps_g, NH)
        gm_ps = mm2(xm, wg_s, ps_g, NH)
        gk_t = work.tile([T, NH], F32, tag="gk")
        nc.vector.tensor_tensor(out=gk_t[:], in0=gk_ps[:], in1=bg_t[:], op=ALU.add)
        nc.scalar.activation(gk_t[:], gk_t[:], AF.Sigmoid)
        gm_t = work.tile([T, NH], F32, tag="gm")
        nc.vector.tensor_tensor(out=gm_t[:], in0=gm_ps[:], in1=bg_t[:], op=ALU.add)
        nc.scalar.activation(gm_t[:], gm_t[:], AF.Sigmoid)

        # ---- attention logit differences per head: d = (Q . (K_kpt - K_meta)) / sqrt(dk)
        # softmax over 2 keys == sigmoid(d);  Y_q = V_meta + a_q * (V_kpt - V_meta)
        invsq = 1.0 / float(np.sqrt(DK))
        tk_t = work.tile([T, C], F32, tag="tk")
        nc.vector.tensor_tensor(out=tk_t[:], in0=qk_t[:], in1=dk_ps[:], op=ALU.mult)
        ak_t = work.tile([T, NH], F32, tag="ak")
        nc.vector.reduce_sum(
            out=ak_t[:], in_=tk_t[:].rearrange("p (h d) -> p h d", d=DK), axis=AX.X
        )
        nc.scalar.activation(ak_t[:], ak_t[:], AF.Sigmoid, scale=invsq)
        tm_t = work.tile([T, C], F32, tag="tm")
        nc.vector.tensor_tensor(out=tm_t[:], in0=qm_t[:], in1=dk_ps[:], op=ALU.mult)
        am_t = work.tile([T, NH], F32, tag="am")
        nc.vector.reduce_sum(
            out=am_t[:], in_=tm_t[:].rearrange("p (h d) -> p h d", d=DK), axis=AX.X
        )
        nc.scalar.activation(am_t[:], am_t[:], AF.Sigmoid, scale=invsq)

        # attended_mean = c0 (x) V_meta' + c1 (x) dV   (per-head coefs, x0.5 baked in E)
        # c0 = g_kpt + g_meta ;  c1 = g_kpt*a_kpt + g_meta*a_meta
        c0_t = work.tile([T, NH], F32, tag="c0")
        nc.vector.tensor_tensor(out=c0_t[:], in0=gk_t[:], in1=gm_t[:], op=ALU.add)
        t1_t = work.tile([T, NH], F32, tag="t1")
        nc.vector.tensor_tensor(out=t1_t[:], in0=gk_t[:], in1=ak_t[:], op=ALU.mult)
        t2_t = work.tile([T, NH], F32, tag="t2")
        nc.vector.tensor_tensor(out=t2_t[:], in0=gm_t[:], in1=am_t[:], op=ALU.mult)
        c1_t = work.tile([T, NH], F32, tag="c1")
        nc.vector.tensor_tensor(out=c1_t[:], in0=t1_t[:], in1=t2_t[:], op=ALU.add)

        # expand per-head coefs to [68, 256] via transpose + matmul with 0.5*E
        def expand(coef_t, tag):
            ct_ps = ps_sm.tile([NH, T], F32, space="PSUM", tag="sm")
            nc.tensor.transpose(out=ct_ps[:], in_=coef_t[:], identity=id68)
            ct_sb = work.tile([NH, T], F32, tag=f"{tag}T")
            nc.vector.tensor_copy(ct_sb[:], ct_ps[:])
            ce_ps = ps_mm.tile([T, C], F32, space="PSUM", tag="mm")
            nc.tensor.matmul(ce_ps[:], ct_sb[:], eh_t[:], start=True, stop=True)
            return ce_ps

        c0e_ps = expand(c0_t, "c0")
        c1e_ps = expand(c1_t, "c1")
        att_t = work.tile([T, C], F32, tag="att")
        nc.vector.tensor_tensor(out=att_t[:], in0=c0e_ps[:], in1=vm_t[:], op=ALU.mult)
        t3_t = work.tile([T, C], F32, tag="t3")
        nc.vector.tensor_tensor(out=t3_t[:], in0=c1e_ps[:], in1=dv_t[:], op=ALU.mult)
        nc.vector.tensor_tensor(out=att_t[:], in0=att_t[:], in1=t3_t[:], op=ALU.add)

        # ---- projected branch: relu(LN(combined @ P1w + P1b)) then (@ P2w later)
        hp_ps = ps_mm.tile([T, C], F32, space="PSUM", tag="mm")
        nc.tensor.matmul(hp_ps[:], xk[0], p1w_s[0][:], start=True, stop=False)
        nc.tensor.matmul(hp_ps[:], xk[1], p1w_s[1][:], start=False, stop=False)
        nc.tensor.matmul(hp_ps[:], xm[0], p1w_s[2][:], start=False, stop=False)
        nc.tensor.matmul(hp_ps[:], xm[1], p1w_s[3][:], start=False, stop=True)
        h_t = work.tile([T, C], F32, tag="h")
        nc.vector.tensor_tensor(out=h_t[:], in0=hp_ps[:], in1=p1b_t[:], op=ALU.add)

        mu_t = work.tile([T, 1], F32, tag="mu")
        nc.vector.reduce_sum(out=mu_t[:], in_=h_t[:], axis=AX.X)
        nc.scalar.mul(mu_t[:], mu_t[:], 1.0 / C)
        nc.vector.tensor_scalar_sub(h_t[:], h_t[:], mu_t[:])
        sq_t = work.tile([T, C], F32, tag="sq")
        nc.vector.tensor_tensor(out=sq_t[:], in0=h_t[:], in1=h_t[:], op=ALU.mult)
        var_t = work.tile([T, 1], F32, tag="var")
        nc.vector.reduce_sum(out=var_t[:], in_=sq_t[:], axis=AX.X)
        std_t = work.tile([T, 1], F32, tag="std")
        eps_t = const.tile([T, 1], F32, tag="eps")
        nc.gpsimd.memset(eps_t[:], float(EPS))
        nc.scalar.activation(std_t[:], var_t[:], AF.Sqrt, bias=eps_t[:], scale=1.0 / C)
        rstd_t = work.tile([T, 1], F32, tag="rstd")
        nc.vector.reciprocal(rstd_t[:], std_t[:])
        nc.vector.tensor_scalar_mul(h_t[:], h_t[:], rstd_t[:])
        nc.vector.tensor_tensor(out=h_t[:], in0=h_t[:], in1=lng_t[:], op=ALU.mult)
        nc.vector.tensor_tensor(out=h_t[:], in0=h_t[:], in1=lnb_t[:], op=ALU.add)
        nc.scalar.activation(h_t[:], h_t[:], AF.Relu)

        # ---- transposes to feature-major for the final contractions
        def transpose_to_sbuf(src_t, tag):
            tiles = []
            for ctile in range(2):
                tp_ps = ps_tp.tile([128, T], F32, space="PSUM", tag="tp")
                nc.tensor.transpose(
                    out=tp_ps[:],
                    in_=src_t[:, 128 * ctile : 128 * (ctile + 1)],
                    identity=id68,
                )
                tp_sb = work.tile([128, T], F32, tag=f"{tag}{ctile}")
                nc.vector.tensor_copy(tp_sb[:], tp_ps[:])
                tiles.append(tp_sb)
            return tiles

        att_T = transpose_to_sbuf(att_t, "attT")
        h_T = transpose_to_sbuf(h_t, "hT")

        # ---- final: att @ Wo + h @ P2w + (bo + P2b), all in one PSUM accumulation
        fin_ps = ps_mm.tile([T, C], F32, space="PSUM", tag="mm")
        nc.tensor.matmul(fin_ps[:], att_T[0][:], wo_s[0][:], start=True, stop=False)
        nc.tensor.matmul(fin_ps[:], att_T[1][:], wo_s[1][:], start=False, stop=False)
        nc.tensor.matmul(fin_ps[:], h_T[0][:], p2w_s[0][:], start=False, stop=False)
        nc.tensor.matmul(fin_ps[:], h_T[1][:], p2w_s[1][:], start=False, stop=True)
        out_t = work.tile([T, C], F32, tag="outt")
        nc.vector.tensor_tensor(out=out_t[:], in0=fin_ps[:], in1=bfin_t[:], op=ALU.add)
        nc.sync.dma_start(out[:], out_t[:])

    # bacc legalization: splits multi-sem waits (TRN2 allows 1 wait/inst),
    # moves matmul waits onto ldweights, allocates registers
    nc.compile()
    return nc


def _get_nc():
    if "nc" not in _NC_CACHE:
        _NC_CACHE["nc"] = _build_program()
    return _NC_CACHE["nc"]


def _gauss_kernel():
    ax = np.arange(-2.0, 3.0, dtype=np.float32)
    xx, yy = np.meshgrid(ax, ax, indexing="ij")
    k = np.exp(-(xx**2 + yy**2) / (2.0 * SIGMA**2)).astype(np.float32)
    return (k / k.sum()).reshape(-1).astype(np.float32)  # (25,)


def _make_in_maps(feature_map, keypoints, meta, Wq, bq, Wk, bk, Wv, bv, Wo, bo,
                  Wg, bg, P1w, P1b, ln_g, ln_b, P2w, P2b):
    f32 = lambda a: np.ascontiguousarray(np.asarray(a), dtype=np.float32)
    feature_map = f32(feature_map)
    keypoints = f32(keypoints)

    # keypoint -> feature-map integer coords.  The oracle reference runs on the
    # in-container jax (axon/neuron platform), whose f32->int32 cast is
    # round-to-nearest-even (np.rint) -- NOT C-style truncation like CPU jax.
    xs = keypoints[..., 0] * np.float32(W / ORIG_W)
    ys = keypoints[..., 1] * np.float32(H / ORIG_H)
    xi = np.clip(np.rint(xs).astype(np.int32), HALF, W - HALF - 1)  # (B, N)
    yi = np.clip(np.rint(ys).astype(np.int32), HALF, H - HALF - 1)

    g = _gauss_kernel()
    gker = np.ascontiguousarray(
        np.broadcast_to(np.tile(g, N), (128, N * PK)), dtype=np.float32
    )

    mt = np.tile(f32(meta), (BPC, 1)).T  # (256, 68)
    metat = np.ascontiguousarray(np.concatenate([mt[:128], mt[128:]], axis=1))

    ehalf = np.zeros((NH, C), dtype=np.float32)
    for h in range(NH):
        ehalf[h, h * DK : (h + 1) * DK] = 0.5

    rep = lambda v, cols: np.ascontiguousarray(
        np.broadcast_to(f32(v), (T, cols)), dtype=np.float32
    )
    common = {
        "gker": gker,
        "metat": metat,
        "wq": f32(Wq), "wk": f32(Wk), "wv": f32(Wv), "wo": f32(Wo),
        "p1w": f32(P1w), "p2w": f32(P2w), "wg": f32(Wg),
        "ehalf": ehalf,
        "bq_r": rep(bq, C), "bv_r": rep(bv, C), "p1b_r": rep(P1b, C),
        "lng_r": rep(ln_g, C), "lnb_r": rep(ln_b, C),
        "bfin_r": rep(f32(bo) + f32(P2b), C), "bg_r": rep(bg, NH),
    }

    in_maps = []
    for m in range(NCORES):
        yc = yi[BPC * m : BPC * (m + 1)].reshape(1, T) - HALF
        xc = xi[BPC * m : BPC * (m + 1)].reshape(1, T) - HALF
        fm_shard = np.ascontiguousarray(feature_map[BPC * m : BPC * (m + 1)]).reshape(
            BPC * C, H, W
        )
        in_maps.append(
            {
                "fm": fm_shard,
                "yoff": np.ascontiguousarray(yc, dtype=np.int32),
                "xoff": np.ascontiguousarray(xc, dtype=np.int32),
                **common,
            }
        )
    return in_maps


def _run(in_maps, **kwargs):
    nc = _get_nc()
    return run_bass_kernel_spmd(nc, in_maps, core_ids=list(range(NCORES)), **kwargs)


def kernel(**inputs) -> np.ndarray:
    in_maps = _make_in_maps(**inputs)
    res = _run(in_maps)
    outs = np.stack([res.results[i]["out"] for i in range(NCORES)])  # (8, T, C)
    return np.ascontiguousarray(outs.reshape(B, N, C))


# revision 14
# speedup vs baseline: 1.1169x; 1.1169x over previous
"""Trainium2 Bass kernel for nn_MetaSpace (keypoint gaussian pooling + meta-space fusion).

Strategy (sharding): data-parallel over batch B: 4 batches per core x 8 cores.
meta / all linear weights replicated.

Gather: each core streams its 4 feature maps channel-half by channel-half into
SBUF with dense, descriptor-optimal HWDGE DMAs, then extracts the 5x5 patch
around each keypoint with register-offset dynamic access patterns on the DVE
(keypoint coords are data -> loaded into engine registers from an int32 input).
Gaussian pooling, the tiny MLP branch and the 2-token gated MHA (softmax over 2
keys done in closed form via sigmoid) all run on-chip in fp32.

Pipeline per core (68 tokens = 4 batches x 17 keypoints):
  dense fm load -> dynamic-AP 5x5 extract -> gaussian pool -> kpt_feat^T [2x128, 68]
  -> PE matmuls vs Wq/Wk/Wv/Wg/P1w (token-major [68,256] PSUM outputs)
  -> closed-form 2-token softmax + headwise gating + token mean
  -> LayerNorm+ReLU branch -> PE transposes -> final Wo/P2w matmuls in one PSUM
  -> out [68, 256] -> DRAM.
"""

import os

import numpy as np
from contextlib import ExitStack

import concourse.bass as bass
import concourse.bacc as bacc
import concourse.tile as tile
import concourse.mybir as mybir
from concourse.bass_utils import run_bass_kernel_spmd
from concourse.masks import make_identity

# ---- problem constants (hardcoded; kernel.py must be self-contained) ----
B, N, C, H, W = 32, 17, 256, 96, 72
NCORES = 8
BPC = B // NCORES          # batches per core = 4
T = BPC * N                # tokens per core = 68
NH = 8                     # heads
DK = C // NH               # 32
KSZ = 5
HALF = 2
SIGMA = 2.0
ORIG_H, ORIG_W = 384, 288
EPS = 1e-5
PK = KSZ * KSZ             # 25
PATCH_F = 2 * T * PK       # patches free size = 3400

F32 = mybir.dt.float32
I32 = mybir.dt.int32
AF = mybir.ActivationFunctionType
ALU = mybir.AluOpType
AX = mybir.AxisListType

_NC_CACHE = {}


def _build_program():
    nc = bacc.Bacc()

    # DRAM I/O
    fm = nc.declare_dram_parameter("fm", [BPC * C, H, W], F32, isOutput=False)
    yoff = nc.declare_dram_parameter("yoff", [1, T], I32, isOutput=False)
    xoff = nc.declare_dram_parameter("xoff", [1, T], I32, isOutput=False)
    gker = nc.declare_dram_parameter("gker", [128, N * PK], F32, isOutput=False)
    metat = nc.declare_dram_parameter("metat", [128, 2 * T], F32, isOutput=False)
    wq = nc.declare_dram_parameter("wq", [C, C], F32, isOutput=False)
    wk = nc.declare_dram_parameter("wk", [C, C], F32, isOutput=False)
    wv = nc.declare_dram_parameter("wv", [C, C], F32, isOutput=False)
    wo = nc.declare_dram_parameter("wo", [C, C], F32, isOutput=False)
    p1w = nc.declare_dram_parameter("p1w", [2 * C, C], F32, isOutput=False)
    p2w = nc.declare_dram_parameter("p2w", [C, C], F32, isOutput=False)
    wg = nc.declare_dram_parameter("wg", [C, NH], F32, isOutput=False)
    ehalf = nc.declare_dram_parameter("ehalf", [NH, C], F32, isOutput=False)
    bq_r = nc.declare_dram_parameter("bq_r", [T, C], F32, isOutput=False)
    bv_r = nc.declare_dram_parameter("bv_r", [T, C], F32, isOutput=False)
    p1b_r = nc.declare_dram_parameter("p1b_r", [T, C], F32, isOutput=False)
    lng_r = nc.declare_dram_parameter("lng_r", [T, C], F32, isOutput=False)
    lnb_r = nc.declare_dram_parameter("lnb_r", [T, C], F32, isOutput=False)
    bfin_r = nc.declare_dram_parameter("bfin_r", [T, C], F32, isOutput=False)
    bg_r = nc.declare_dram_parameter("bg_r", [T, NH], F32, isOutput=False)
    out = nc.declare_dram_parameter("out", [T, C], F32, isOutput=True)
    dbg = os.environ.get("K_DEBUG") == "1"
    if dbg:
        d_xt = nc.declare_dram_parameter("d_xt", [128, 2 * T], F32, isOutput=True)
        d_p0 = nc.declare_dram_parameter("d_p0", [128, PATCH_F], F32, isOutput=True)

    with ExitStack() as ctx:
        tc = ctx.enter_context(tile.TileContext(nc))
        const = ctx.enter_context(tc.tile_pool(name="const", bufs=1))
        work = ctx.enter_context(tc.tile_pool(name="work", bufs=1))
        fmpool = ctx.enter_context(tc.tile_pool(name="fmpool", bufs=2))
        # PSUM pools: [68,256] matmul outs rotate in 3 banks; gates need 2 live;
        # transposes rotate in 2; [8,68] small transposes in 1.  3+2+2+1 = 8 banks.
        ps_mm = ctx.enter_context(tc.tile_pool(name="ps_mm", bufs=3, space="PSUM"))
        ps_g = ctx.enter_context(tc.tile_pool(name="ps_g", bufs=2, space="PSUM"))
        ps_tp = ctx.enter_context(tc.tile_pool(name="ps_tp", bufs=2, space="PSUM"))
        ps_sm = ctx.enter_context(tc.tile_pool(name="ps_sm", bufs=1, space="PSUM"))

        # ---- stage constants / weights (HWDGE), issue early so they overlap fm loads
        yx_t = const.tile([1, 2 * T], I32, tag="yx")
        nc.sync.dma_start(yx_t[:, :T], yoff[:])
        nc.sync.dma_start(yx_t[:, T:], xoff[:])
        g_t = const.tile([128, N * PK], F32, tag="gker")
        nc.sync.dma_start(g_t[:], gker[:])
        mt_t = const.tile([128, 2 * T], F32, tag="metat")
        nc.sync.dma_start(mt_t[:], metat[:])

        def load_slices(dram, rows, cols, tagbase):
            tiles = []
            for k in range(rows // 128):
                t_ = const.tile([128, cols], F32, tag=f"{tagbase}{k}")
                nc.sync.dma_start(t_[:], dram[128 * k : 128 * (k + 1), :])
                tiles.append(t_)
            return tiles

        wq_s = load_slices(wq, C, C, "wq")
        wk_s = load_slices(wk, C, C, "wk")
        wv_s = load_slices(wv, C, C, "wv")
        wo_s = load_slices(wo, C, C, "wo")
        p1w_s = load_slices(p1w, 2 * C, C, "p1w")
        p2w_s = load_slices(p2w, C, C, "p2w")
        wg_s = load_slices(wg, C, NH, "wg")

        eh_t = const.tile([NH, C], F32, tag="ehalf")
        nc.sync.dma_start(eh_t[:], ehalf[:])

        def load_rep(dram, cols, tag):
            t_ = const.tile([T, cols], F32, tag=tag)
            nc.sync.dma_start(t_[:], dram[:])
            return t_

        bq_t = load_rep(bq_r, C, "bq")
        bv_t = load_rep(bv_r, C, "bv")
        p1b_t = load_rep(p1b_r, C, "p1b")
        lng_t = load_rep(lng_r, C, "lng")
        lnb_t = load_rep(lnb_r, C, "lnb")
        bfin_t = load_rep(bfin_r, C, "bfin")
        bg_t = load_rep(bg_r, NH, "bg")

        id_t = const.tile([128, 128], F32, tag="ident")
        make_identity(nc, id_t[:])
        id68 = id_t[:T, :T]

        # ---- dense fm stream + dynamic-AP 5x5 extraction + gaussian pooling
        # patches[:, ct*T + t, :] = fm[bl, ctile_c, y_t-2:y_t+3, x_t-2:x_t+3]
        patches = const.tile([128, PATCH_F], F32, tag="patches")
        xt = const.tile([128, 2 * T], F32, tag="xt")
        for bl in range(BPC):
            fmt = []
            for ct in range(2):
                ft = fmpool.tile([128, H * W], F32, tag=f"fm{ct}")
                nc.sync.dma_start(
                    ft[:],
                    fm[bl * C + ct * 128 : bl * C + ct * 128 + 128, :, :].rearrange(
                        "c y x -> c (y x)"
                    ),
                )
                fmt.append(ft)
            # batched coordinate loads: one TENSOR_LOAD per engine per coord set
            ysl = yx_t[0:1, bl * N : (bl + 1) * N]
            xsl = yx_t[0:1, T + bl * N : T + (bl + 1) * N]
            DVE = mybir.EngineType.DVE
            ACT = mybir.EngineType.Activation
            coords = {}
            for eng in (DVE, ACT):
                _, yv = nc.values_load_multi_w_load_instructions(
                    ysl, engines=(eng,), min_val=0, max_val=H - KSZ,
                    skip_runtime_bounds_check=True,
                )
                _, xv = nc.values_load_multi_w_load_instructions(
                    xsl, engines=(eng,), min_val=0, max_val=W - KSZ,
                    skip_runtime_bounds_check=True,
                )
                coords[eng] = (yv, xv)
            for n in range(N):
                t = bl * N + n
                # ct0 extraction on DVE, ct1 on ACT (parallel engines)
                for ct, eng in ((0, DVE), (1, ACT)):
                    yv, xv = coords[eng]
                    view = fmt[ct][:].rearrange("p (y x) -> p y x", x=W)
                    dst = patches[
                        :, (ct * T + t) * PK : (ct * T + t + 1) * PK
                    ].rearrange("p (a b) -> p a b", b=KSZ)
                    src = view[:, bass.ds(yv[n], KSZ), bass.ds(xv[n], KSZ)]
                    if eng is DVE:
                        nc.vector.tensor_copy(dst, src)
                    else:
                        nc.scalar.copy(dst, src)
            # gaussian pooling for this batch, both channel halves
            for ct in range(2):
                csl = slice((ct * T + bl * N) * PK, (ct * T + bl * N + N) * PK)
                wt = work.tile([128, N * PK], F32, tag="wtmp")
                nc.vector.tensor_tensor(
                    out=wt[:], in0=patches[:, csl], in1=g_t[:], op=ALU.mult
                )
                nc.vector.reduce_sum(
                    out=xt[:, ct * T + bl * N : ct * T + bl * N + N],
                    in_=wt[:].rearrange("p (k r) -> p k r", r=PK),
                    axis=AX.X,
                )

        if dbg:
            nc.sync.dma_start(d_xt[:], xt[:])
            nc.sync.dma_start(d_p0[:], patches[:])

        # lhsT slices: kpt tokens (xt) and meta tokens (mt_t), per channel-half
        xk = [xt[:, :T], xt[:, T:]]
        xm = [mt_t[:, :T], mt_t[:, T:]]

        def mm2(lhs_pair, rhs_pair, pool, n):
            ps = pool.tile([T, n], F32, space="PSUM", tag="mm")
            nc.tensor.matmul(ps[:], lhs_pair[0], rhs_pair[0][:], start=True, stop=False)
            nc.tensor.matmul(ps[:], lhs_pair[1], rhs_pair[1][:], start=False, stop=True)
            return ps

        # negated meta activations let PE accumulate K/V *differences* in PSUM
        # (walrus forbids reading two PSUM operands in one DVE op)
        xmn_t = const.tile([128, 2 * T], F32, tag="xmn")
        nc.scalar.mul(xmn_t[:], mt_t[:], -1.0)
        xmn = [xmn_t[:, :T], xmn_t[:, T:]]

        def mm4(lhs_a, lhs_b, rhs_pair, pool, n):
            ps = pool.tile([T, n], F32, space="PSUM", tag="mm")
            nc.tensor.matmul(ps[:], lhs_a[0], rhs_pair[0][:], start=True, stop=False)
            nc.tensor.matmul(ps[:], lhs_a[1], rhs_pair[1][:], start=False, stop=False)
            nc.tensor.matmul(ps[:], lhs_b[0], rhs_pair[0][:], start=False, stop=False)
            nc.tensor.matmul(ps[:], lhs_b[1], rhs_pair[1][:], start=False, stop=True)
            return ps

        # ---- K/V first so their PSUM banks recycle fastest
        dk_ps = mm4(xk, xmn, wk_s, ps_mm, C)  # K_kpt - K_meta, stays in PSUM

        dv_ps = mm4(xk, xmn, wv_s, ps_mm, C)  # V_kpt - V_meta
        dv_t = work.tile([T, C], F32, tag="dv")
        nc.vector.tensor_copy(dv_t[:], dv_ps[:])
        vm_ps = mm2(xm, wv_s, ps_mm, C)
        vm_t = work.tile([T, C], F32, tag="vm")
        nc.vector.tensor_tensor(out=vm_t[:], in0=vm_ps[:], in1=bv_t[:], op=ALU.add)

        qk_ps = mm2(xk, wq_s, ps_mm, C)
        qk_t = work.tile([T, C], F32, tag="qk")
        nc.vector.tensor_tensor(out=qk_t[:], in0=qk_ps[:], in1=bq_t[:], op=ALU.add)
        qm_ps = mm2(xm, wq_s, ps_mm, C)
        qm_t = work.tile([T, C], F32, tag="qm")
        nc.vector.tensor_tensor(out=qm_t[:], in0=qm_ps[:], in1=bq_t[:], op=ALU.add)

        # ---- gates: sigmoid(x @ Wg + bg)  [68, 8]
        gk_ps = mm2(xk, wg_s, ps_g, NH)
        gm_ps = mm2(xm, wg_s, ps_g, NH)
        gk_t = work.tile([T, NH], F32, tag="gk")
        nc.vector.tensor_tensor(out=gk_t[:], in0=gk_ps[:], in1=bg_t[:], op=ALU.add)
        nc.scalar.activation(gk_t[:], gk_t[:], AF.Sigmoid)
        gm_t = work.tile([T, NH], F32, tag="gm")
        nc.vector.tensor_tensor(out=gm_t[:], in0=gm_ps[:], in1=bg_t[:], op=ALU.add)
        nc.scalar.activation(gm_t[:], gm_t[:], AF.Sigmoid)

        # ---- attention logit differences per head: d = (Q . (K_kpt - K_meta)) / sqrt(dk)
        # softmax over 2 keys == sigmoid(d);  Y_q = V_meta + a_q * (V_kpt - V_meta)
        invsq = 1.0 / float(np.sqrt(DK))
        tk_t = work.tile([T, C], F32, tag="tk")
        nc.vector.tensor_tensor(out=tk_t[:], in0=qk_t[:], in1=dk_ps[:], op=ALU.mult)
        ak_t = work.tile([T, NH], F32, tag="ak")
        nc.vector.reduce_sum(
            out=ak_t[:], in_=tk_t[:].rearrange("p (h d) -> p h d", d=DK), axis=AX.X
        )
        nc.scalar.activation(ak_t[:], ak_t[:], AF.Sigmoid, scale=invsq)
        tm_t = work.tile([T, C], F32, tag="tm")
        nc.vector.tensor_tensor(out=tm_t[:], in0=qm_t[:], in1=dk_ps[:], op=ALU.mult)
        am_t = work.tile([T, NH], F32, tag="am")
        nc.vector.reduce_sum(
            out=am_t[:], in_=tm_t[:].rearrange("p (h d) -> p h d", d=DK), axis=AX.X
        )
        nc.scalar.activation(am_t[:], am_t[:], AF.Sigmoid, scale=invsq)

        # attended_mean = c0 (x) V_meta' + c1 (x) dV   (per-head coefs, x0.5 baked in E)
        # c0 = g_kpt + g_meta ;  c1 = g_kpt*a_kpt + g_meta*a_meta
        c0_t = work.tile([T, NH], F32, tag="c0")
        nc.vector.tensor_tensor(out=c0_t[:], in0=gk_t[:], in1=gm_t[:], op=ALU.add)
        t1_t = work.tile([T, NH], F32, tag="t1")
        nc.vector.tensor_tensor(out=t1_t[:], in0=gk_t[:], in1=ak_t[:], op=ALU.mult)
        t2_t = work.tile([T, NH], F32, tag="t2")
        nc.vector.tensor_tensor(out=t2_t[:], in0=gm_t[:], in1=am_t[:], op=ALU.mult)
        c1_t = work.tile([T, NH], F32, tag="c1")
        nc.vector.tensor_tensor(out=c1_t[:], in0=t1_t[:], in1=t2_t[:], op=ALU.add)

        # expand per-head coefs to [68, 256] via transpose + matmul with 0.5*E
        def expand(coef_t, tag):
            ct_ps = ps_sm.tile([NH, T], F32, space="PSUM", tag="sm")
            nc.tensor.transpose(out=ct_ps[:], in_=coef_t[:], identity=id68)
            ct_sb = work.tile([NH, T], F32, tag=f"{tag}T")
            nc.vector.tensor_copy(ct_sb[:], ct_ps[:])
            ce_ps = ps_mm.tile([T, C], F32, space="PSUM", tag="mm")
            nc.tensor.matmul(ce_ps[:], ct_sb[:], eh_t[:], start=True, stop=True)
            return ce_ps

        c0e_ps = expand(c0_t, "c0")
        c1e_ps = expand(c1_t, "c1")
        att_t = work.tile([T, C], F32, tag="att")
        nc.vector.tensor_tensor(out=att_t[:], in0=c0e_ps[:], in1=vm_t[:], op=ALU.mult)
        t3_t = work.tile([T, C], F32, tag="t3")
        nc.vector.tensor_tensor(out=t3_t[:], in0=c1e_ps[:], in1=dv_t[:], op=ALU.mult)
        nc.vector.tensor_tensor(out=att_t[:], in0=att_t[:], in1=t3_t[:], op=ALU.add)

        # ---- projected branch: relu(LN(combined @ P1w + P1b)) then (@ P2w later)
        hp_ps = ps_mm.tile([T, C], F32, space="PSUM", tag="mm")
        nc.tensor.matmul(hp_ps[:], xk[0], p1w_s[0][:], start=True, stop=False)
        nc.tensor.matmul(hp_ps[:], xk[1], p1w_s[1][:], start=False, stop=False)
        nc.tensor.matmul(hp_ps[:], xm[0], p1w_s[2][:], start=False, stop=False)
        nc.tensor.matmul(hp_ps[:], xm[1], p1w_s[3][:], start=False, stop=True)
        h_t = work.tile([T, C], F32, tag="h")
        nc.vector.tensor_tensor(out=h_t[:], in0=hp_ps[:], in1=p1b_t[:], op=ALU.add)

        mu_t = work.tile([T, 1], F32, tag="mu")
        nc.vector.reduce_sum(out=mu_t[:], in_=h_t[:], axis=AX.X)
        nc.scalar.mul(mu_t[:], mu_t[:], 1.0 / C)
        nc.vector.tensor_scalar_sub(h_t[:], h_t[:], mu_t[:])
        sq_t = work.tile([T, C], F32, tag="sq")
        nc.vector.tensor_tensor(out=sq_t[:], in0=h_t[:], in1=h_t[:], op=ALU.mult)
        var_t = work.tile([T, 1], F32, tag="var")
        nc.vector.reduce_sum(out=var_t[:], in_=sq_t[:], axis=AX.X)
        std_t = work.tile([T, 1], F32, tag="std")
        eps_t = const.tile([T, 1], F32, tag="eps")
        nc.gpsimd.memset(eps_t[:], float(EPS))
        nc.scalar.activation(std_t[:], var_t[:], AF.Sqrt, bias=eps_t[:], scale=1.0 / C)
        rstd_t = work.tile([T, 1], F32, tag="rstd")
        nc.vector.reciprocal(rstd_t[:], std_t[:])
        nc.vector.tensor_scalar_mul(h_t[:], h_t[:], rstd_t[:])
        nc.vector.tensor_tensor(out=h_t[:], in0=h_t[:], in1=lng_t[:], op=ALU.mult)
        nc.vector.tensor_tensor(out=h_t[:], in0=h_t[:], in1=lnb_t[:], op=ALU.add)
        nc.scalar.activation(h_t[:], h_t[:], AF.Relu)

        # ---- transposes to feature-major for the final contractions
        def transpose_to_sbuf(src_t, tag):
            tiles = []
            for ctile in range(2):
                tp_ps = ps_tp.tile([128, T], F32, space="PSUM", tag="tp")
                nc.tensor.transpose(
                    out=tp_ps[:],
                    in_=src_t[:, 128 * ctile : 128 * (ctile + 1)],
                    identity=id68,
                )
                tp_sb = work.tile([128, T], F32, tag=f"{tag}{ctile}")
                nc.vector.tensor_copy(tp_sb[:], tp_ps[:])
                tiles.append(tp_sb)
            return tiles

        att_T = transpose_to_sbuf(att_t, "attT")
        h_T = transpose_to_sbuf(h_t, "hT")

        # ---- final: att @ Wo + h @ P2w + (bo + P2b), all in one PSUM accumulation
        fin_ps = ps_mm.tile([T, C], F32, space="PSUM", tag="mm")
        nc.tensor.matmul(fin_ps[:], att_T[0][:], wo_s[0][:], start=True, stop=False)
        nc.tensor.matmul(fin_ps[:], att_T[1][:], wo_s[1][:], start=False, stop=False)
        nc.tensor.matmul(fin_ps[:], h_T[0][:], p2w_s[0][:], start=False, stop=False)
        nc.tensor.matmul(fin_ps[:], h_T[1][:], p2w_s[1][:], start=False, stop=True)
        out_t = work.tile([T, C], F32, tag="outt")
        nc.vector.tensor_tensor(out=out_t[:], in0=fin_ps[:], in1=bfin_t[:], op=ALU.add)
        nc.sync.dma_start(out[:], out_t[:])

    # bacc legalization: splits multi-sem waits (TRN2 allows 1 wait/inst),
    # moves matmul waits onto ldweights, allocates registers
    nc.compile()
    return nc


def _get_nc():
    if "nc" not in _NC_CACHE:
        _NC_CACHE["nc"] = _build_program()
    return _NC_CACHE["nc"]


def _gauss_kernel():
    ax = np.arange(-2.0, 3.0, dtype=np.float32)
    xx, yy = np.meshgrid(ax, ax, indexing="ij")
    k = np.exp(-(xx**2 + yy**2) / (2.0 * SIGMA**2)).astype(np.float32)
    return (k / k.sum()).reshape(-1).astype(np.float32)  # (25,)


def _make_in_maps(feature_map, keypoints, meta, Wq, bq, Wk, bk, Wv, bv, Wo, bo,
                  Wg, bg, P1w, P1b, ln_g, ln_b, P2w, P2b):
    f32 = lambda a: np.ascontiguousarray(np.asarray(a), dtype=np.float32)
    feature_map = f32(feature_map)
    keypoints = f32(keypoints)

    # keypoint -> feature-map integer coords.  The oracle reference runs on the
    # in-container jax (axon/neuron platform), whose f32->int32 cast is
    # round-to-nearest-even (np.rint) -- NOT C-style truncation like CPU jax.
    xs = keypoints[..., 0] * np.float32(W / ORIG_W)
    ys = keypoints[..., 1] * np.float32(H / ORIG_H)
    xi = np.clip(np.rint(xs).astype(np.int32), HALF, W - HALF - 1)  # (B, N)
    yi = np.clip(np.rint(ys).astype(np.int32), HALF, H - HALF - 1)

    g = _gauss_kernel()
    gker = np.ascontiguousarray(
        np.broadcast_to(np.tile(g, N), (128, N * PK)), dtype=np.float32
    )

    mt = np.tile(f32(meta), (BPC, 1)).T  # (256, 68)
    metat = np.ascontiguousarray(np.concatenate([mt[:128], mt[128:]], axis=1))

    ehalf = np.zeros((NH, C), dtype=np.float32)
    for h in range(NH):
        ehalf[h, h * DK : (h + 1) * DK] = 0.5

    rep = lambda v, cols: np.ascontiguousarray(
        np.broadcast_to(f32(v), (T, cols)), dtype=np.float32
    )
    common = {
        "gker": gker,
        "metat": metat,
        "wq": f32(Wq), "wk": f32(Wk), "wv": f32(Wv), "wo": f32(Wo),
        "p1w": f32(P1w), "p2w": f32(P2w), "wg": f32(Wg),
        "ehalf": ehalf,
        "bq_r": rep(bq, C), "bv_r": rep(bv, C), "p1b_r": rep(P1b, C),
        "lng_r": rep(ln_g, C), "lnb_r": rep(ln_b, C),
        "bfin_r": rep(f32(bo) + f32(P2b), C), "bg_r": rep(bg, NH),
    }

    in_maps = []
    for m in range(NCORES):
        yc = yi[BPC * m : BPC * (m + 1)].reshape(1, T) - HALF
        xc = xi[BPC * m : BPC * (m + 1)].reshape(1, T) - HALF
        fm_shard = np.ascontiguousarray(feature_map[BPC * m : BPC * (m + 1)]).reshape(
            BPC * C, H, W
        )
        in_maps.append(
            {
                "fm": fm_shard,
                "yoff": np.ascontiguousarray(yc, dtype=np.int32),
                "xoff": np.ascontiguousarray(xc, dtype=np.int32),
                **common,
            }
        )
    return in_maps


def _run(in_maps, **kwargs):
    nc = _get_nc()
    return run_bass_kernel_spmd(nc, in_maps, core_ids=list(range(NCORES)), **kwargs)


def kernel(**inputs) -> np.ndarray:
    in_maps = _make_in_maps(**inputs)
    res = _run(in_maps)
    outs = np.stack([res.results[i]["out"] for i in range(NCORES)])  # (8, T, C)
    return np.ascontiguousarray(outs.reshape(B, N, C))


# revision 18
# speedup vs baseline: 1.4134x; 1.2655x over previous
"""Trainium2 Bass kernel for nn_MetaSpace (keypoint gaussian pooling + meta-space fusion).

Strategy (sharding): data-parallel over batch B: 4 batches per core x 8 cores.
meta / all linear weights replicated.

Gather: each core streams its 4 feature maps channel-half by channel-half into
SBUF with dense, descriptor-optimal HWDGE DMAs, then extracts the 5x5 patch
around each keypoint with register-offset dynamic access patterns on the DVE
(keypoint coords are data -> loaded into engine registers from an int32 input).
Gaussian pooling, the tiny MLP branch and the 2-token gated MHA (softmax over 2
keys done in closed form via sigmoid) all run on-chip in fp32.

Pipeline per core (68 tokens = 4 batches x 17 keypoints):
  dense fm load -> dynamic-AP 5x5 extract -> gaussian pool -> kpt_feat^T [2x128, 68]
  -> PE matmuls vs Wq/Wk/Wv/Wg/P1w (token-major [68,256] PSUM outputs)
  -> closed-form 2-token softmax + headwise gating + token mean
  -> LayerNorm+ReLU branch -> PE transposes -> final Wo/P2w matmuls in one PSUM
  -> out [68, 256] -> DRAM.
"""

import os

import numpy as np
from contextlib import ExitStack

import concourse.bass as bass
import concourse.bacc as bacc
import concourse.tile as tile
import concourse.mybir as mybir
from concourse.bass_utils import run_bass_kernel_spmd
from concourse.masks import make_identity

# ---- problem constants (hardcoded; kernel.py must be self-contained) ----
B, N, C, H, W = 32, 17, 256, 96, 72
NCORES = 8
BPC = B // NCORES          # batches per core = 4
T = BPC * N                # tokens per core = 68
NH = 8                     # heads
DK = C // NH               # 32
KSZ = 5
HALF = 2
SIGMA = 2.0
ORIG_H, ORIG_W = 384, 288
EPS = 1e-5
PK = KSZ * KSZ             # 25
PATCH_F = 2 * T * PK       # patches free size = 3400
PK128 = 4417               # packed [128, *] constant blob width
PK68 = 6 * C + NH          # packed [68, *] bias blob width

F32 = mybir.dt.float32
I32 = mybir.dt.int32
AF = mybir.ActivationFunctionType
ALU = mybir.AluOpType
AX = mybir.AxisListType

_NC_CACHE = {}


def _build_program():
    nc = bacc.Bacc()

    # Token-major tensors use a PADDED row layout: token (bl, n) lives on
    # partition 64*(bl//2) + 17*(bl%2) + n.  Two 34-token halves at partition
    # bases {0, 64} -- the only bases PE matmul outputs and sliced SBUF
    # accesses accept (34 rows from base 64 is legal; 4x 17-row groups at
    # base 96 is not).  Pad rows are zeroed once and ignored by the host.
    fm = nc.declare_dram_parameter("fm", [BPC * C, H * W], F32, isOutput=False)
    foff = nc.declare_dram_parameter("foff", [1, T], I32, isOutput=False)
    pk = nc.declare_dram_parameter("pk", [128, PK128], F32, isOutput=False)
    p68 = nc.declare_dram_parameter("p68", [128, PK68], F32, isOutput=False)
    out = nc.declare_dram_parameter("out", [128, C], F32, isOutput=True)
    dbg = os.environ.get("K_DEBUG") == "1"
    if dbg:
        d_xt = nc.declare_dram_parameter("d_xt", [128, 2 * T], F32, isOutput=True)

    DVE = mybir.EngineType.DVE
    ACT = mybir.EngineType.Activation
    HT = 2 * N  # tokens per half = 34

    with ExitStack() as ctx:
        tc = ctx.enter_context(tile.TileContext(nc))
        const = ctx.enter_context(tc.tile_pool(name="const", bufs=1))
        work = ctx.enter_context(tc.tile_pool(name="work", bufs=1))
        fmpool = ctx.enter_context(tc.tile_pool(name="fmpool", bufs=2))
        # PSUM budget (8 banks): 5 keypoint-side accumulators + 2 rotating
        # whole-tensor banks (early meta matmuls, expands, transposes, final)
        # + 1 small-transpose bank.
        ps_mm = ctx.enter_context(tc.tile_pool(name="ps_mm", bufs=2, space="PSUM"))
        ps_acc = ctx.enter_context(tc.tile_pool(name="ps_acc", bufs=5, space="PSUM"))
        ps_sm = ctx.enter_context(tc.tile_pool(name="ps_sm", bufs=1, space="PSUM"))

        # ---- fm DMAs FIRST so the HBM stream starts immediately
        fmt = {}
        for bl in range(BPC):
            for ct in range(2):
                ft = fmpool.tile([128, H * W + 360], F32, tag=f"fm{ct}")
                nc.sync.dma_start(
                    ft[:, : H * W],
                    fm[bl * C + ct * 128 : bl * C + ct * 128 + 128, :],
                )
                fmt[(bl, ct)] = ft

        # ---- tiny staging (3 DMAs) behind the fm issues
        fo_t = const.tile([1, T], I32, tag="fo")
        nc.sync.dma_start(fo_t[:], foff[:])
        pk_t = const.tile([128, PK128], F32, tag="pk")
        nc.sync.dma_start(pk_t[:], pk[:])
        p68_t = const.tile([128, PK68], F32, tag="p68")
        nc.sync.dma_start(p68_t[:], p68[:])

        def w2(off):
            return [pk_t[:, off : off + C], pk_t[:, off + C : off + 2 * C]]

        wq_s, wk_s, wv_s, wo_s = w2(0), w2(512), w2(1024), w2(1536)
        p1w_s = [pk_t[:, 2048 + k * C : 2048 + (k + 1) * C] for k in range(4)]
        p2w_s = w2(3072)
        wg_s = [pk_t[:, 3584:3592], pk_t[:, 3592:3600]]
        xm = [pk_t[:, 3600 : 3600 + T], pk_t[:, 3600 + T : 3600 + 2 * T]]
        gkv = pk_t[:, 3736:4161]
        ehv = pk_t[:NH, 4161:4417]
        bq_t = p68_t[:, 0:C]
        bv_t = p68_t[:, C : 2 * C]
        p1b_t = p68_t[:, 2 * C : 3 * C]
        lng_t = p68_t[:, 3 * C : 4 * C]
        lnb_t = p68_t[:, 4 * C : 5 * C]
        bfin_t = p68_t[:, 5 * C : 6 * C]
        bg_t = p68_t[:, 6 * C : 6 * C + NH]

        id_t = const.tile([128, 128], F32, tag="ident")
        make_identity(nc, id_t[:])
        eps_t = const.tile([128, 1], F32, tag="eps")
        nc.gpsimd.memset(eps_t[:], float(EPS))

        def hsl(h):
            return slice(64 * h, 64 * h + HT)

        def zwork(shape, tag):
            t_ = work.tile(shape, F32, tag=tag)
            nc.gpsimd.memset(t_[:], 0.0)
            return t_

        # SBUF tiles written per-half; pad rows zeroed once (keeps every
        # downstream whole-tile op NaN-free)
        km_t = zwork([128, C], "km")
        vmr_t = zwork([128, C], "vmr")
        qm_t = zwork([128, C], "qm")
        gm_t = zwork([128, NH], "gm")
        h_t = zwork([128, C], "h")
        dk_t = zwork([128, C], "dk")
        dv_t = zwork([128, C], "dv")
        qk_t = zwork([128, C], "qk")
        gk_t = zwork([128, NH], "gk")

        # ---- EARLY: meta-token matmuls fill the pre-stream dead window
        def meta_mm(rhs_pair, n):
            ps = ps_mm.tile([128, n], F32, space="PSUM", tag="mm")
            for h_ in range(2):
                # halves hit disjoint partitions of one bank: safe on HW, but
                # the sim group tracker drops the partition base -> skip check
                nc.tensor.matmul(ps[hsl(h_), :], xm[0][:, 34 * h_ : 34 * h_ + HT],
                                 rhs_pair[0][:], start=True, stop=False,
                                 skip_group_check=True)
                nc.tensor.matmul(ps[hsl(h_), :], xm[1][:, 34 * h_ : 34 * h_ + HT],
                                 rhs_pair[1][:], start=False, stop=True,
                                 skip_group_check=True)
            return ps

        def psum_to_halves(ps, dst, bias, op):
            for h_ in range(2):
                nc.vector.tensor_tensor(
                    out=dst[hsl(h_), :], in0=ps[hsl(h_), :],
                    in1=bias[hsl(h_), :], op=op,
                )

        km_ps = meta_mm(wk_s, C)
        for h_ in range(2):
            nc.vector.tensor_copy(km_t[hsl(h_), :], km_ps[hsl(h_), :])
        vm_ps = meta_mm(wv_s, C)
        for h_ in range(2):
            nc.vector.tensor_copy(vmr_t[hsl(h_), :], vm_ps[hsl(h_), :])
        vm_t = work.tile([128, C], F32, tag="vm")
        nc.vector.tensor_tensor(out=vm_t[:], in0=vmr_t[:], in1=bv_t, op=ALU.add)
        qm_ps = meta_mm(wq_s, C)
        psum_to_halves(qm_ps, qm_t, bq_t, ALU.add)
        gm_ps = meta_mm(wg_s, NH)
        psum_to_halves(gm_ps, gm_t, bg_t, ALU.add)
        nc.scalar.activation(gm_t[:], gm_t[:], AF.Sigmoid)
        hm_ps = meta_mm(p1w_s[2:4], C)
        psum_to_halves(hm_ps, h_t, p1b_t, ALU.add)

        # keypoint-side PSUM accumulators, written per half during the stream
        kk_acc = ps_acc.tile([128, C], F32, space="PSUM", tag="acc")
        vk_acc = ps_acc.tile([128, C], F32, space="PSUM", tag="acc")
        qk_acc = ps_acc.tile([128, C], F32, space="PSUM", tag="acc")
        gk_acc = ps_acc.tile([128, NH], F32, space="PSUM", tag="acc")
        hp_acc = ps_acc.tile([128, C], F32, space="PSUM", tag="acc")

        # ---- STREAM: per batch extract 5x5 windows (DVE=ct0, ACT=ct1) and
        # gaussian-pool; per half run the keypoint-side matmuls + combines
        patches = const.tile([128, PATCH_F], F32, tag="patches")
        xt = const.tile([128, 2 * T], F32, tag="xt")
        for bl in range(BPC):
            osl = fo_t[0:1, bl * N : (bl + 1) * N]
            coords = {}
            for eng in (DVE, ACT):
                _, ov = nc.values_load_multi_w_load_instructions(
                    osl, engines=(eng,), min_val=0,
                    max_val=(H - KSZ) * W + W - KSZ,
                    skip_runtime_bounds_check=True,
                )
                coords[eng] = ov
            for n in range(N):
                t = bl * N + n
                for ct, eng in ((0, DVE), (1, ACT)):
                    ov = coords[eng]
                    src = (
                        fmt[(bl, ct)][:][:, bass.ds(ov[n], 360)]
                        .rearrange("p (a b) -> p a b", b=W)[:, :, 0:KSZ]
                    )
                    dst = patches[
                        :, (ct * T + t) * PK : (ct * T + t + 1) * PK
                    ].rearrange("p (a b) -> p a b", b=KSZ)
                    if eng is DVE:
                        nc.vector.tensor_copy(dst, src)
                    else:
                        nc.scalar.copy(dst, src)
            for ct in range(2):
                csl = slice((ct * T + bl * N) * PK, (ct * T + bl * N + N) * PK)
                wt = work.tile([128, N * PK], F32, tag="wtmp")
                nc.vector.tensor_tensor(
                    out=wt[:], in0=patches[:, csl], in1=gkv, op=ALU.mult
                )
                nc.vector.reduce_sum(
                    out=xt[:, ct * T + bl * N : ct * T + bl * N + N],
                    in_=wt[:].rearrange("p (k r) -> p k r", r=PK),
                    axis=AX.X,
                )
            if bl % 2 == 0:
                continue
            # both batches of this half are pooled: matmuls + combines
            h_ = bl // 2
            sl = hsl(h_)
            xkh = [xt[:, 34 * h_ : 34 * h_ + HT],
                   xt[:, T + 34 * h_ : T + 34 * h_ + HT]]

            def half_mm(acc, rhs_pair):
                nc.tensor.matmul(acc[sl, :], xkh[0], rhs_pair[0][:],
                                 start=True, stop=False, skip_group_check=True)
                nc.tensor.matmul(acc[sl, :], xkh[1], rhs_pair[1][:],
                                 start=False, stop=True, skip_group_check=True)

            half_mm(kk_acc, wk_s)
            nc.vector.tensor_tensor(out=dk_t[sl, :], in0=kk_acc[sl, :],
                                    in1=km_t[sl, :], op=ALU.subtract)
            half_mm(vk_acc, wv_s)
            nc.vector.tensor_tensor(out=dv_t[sl, :], in0=vk_acc[sl, :],
                                    in1=vmr_t[sl, :], op=ALU.subtract)
            half_mm(qk_acc, wq_s)
            nc.vector.tensor_tensor(out=qk_t[sl, :], in0=qk_acc[sl, :],
                                    in1=bq_t[sl, :], op=ALU.add)
            half_mm(gk_acc, wg_s)
            nc.vector.tensor_tensor(out=gk_t[sl, :], in0=gk_acc[sl, :],
                                    in1=bg_t[sl, :], op=ALU.add)
            half_mm(hp_acc, p1w_s[0:2])
            nc.vector.tensor_tensor(out=h_t[sl, :], in0=hp_acc[sl, :],
                                    in1=h_t[sl, :], op=ALU.add)

        if dbg:
            nc.sync.dma_start(d_xt[:], xt[:])

        # ---- whole-tensor tail (everything now in SBUF, padded layout)
        nc.scalar.activation(gk_t[:], gk_t[:], AF.Sigmoid)

        invsq = 1.0 / float(np.sqrt(DK))
        tk_t = work.tile([128, C], F32, tag="tk")
        nc.vector.tensor_tensor(out=tk_t[:], in0=qk_t[:], in1=dk_t[:], op=ALU.mult)
        ak_t = work.tile([128, NH], F32, tag="ak")
        nc.vector.reduce_sum(
            out=ak_t[:], in_=tk_t[:].rearrange("p (h d) -> p h d", d=DK), axis=AX.X
        )
        nc.scalar.activation(ak_t[:], ak_t[:], AF.Sigmoid, scale=invsq)
        tm_t = work.tile([128, C], F32, tag="tm")
        nc.vector.tensor_tensor(out=tm_t[:], in0=qm_t[:], in1=dk_t[:], op=ALU.mult)
        am_t = work.tile([128, NH], F32, tag="am")
        nc.vector.reduce_sum(
            out=am_t[:], in_=tm_t[:].rearrange("p (h d) -> p h d", d=DK), axis=AX.X
        )
        nc.scalar.activation(am_t[:], am_t[:], AF.Sigmoid, scale=invsq)

        c0_t = work.tile([128, NH], F32, tag="c0")
        nc.vector.tensor_tensor(out=c0_t[:], in0=gk_t[:], in1=gm_t[:], op=ALU.add)
        t1_t = work.tile([128, NH], F32, tag="t1")
        nc.vector.tensor_tensor(out=t1_t[:], in0=gk_t[:], in1=ak_t[:], op=ALU.mult)
        t2_t = work.tile([128, NH], F32, tag="t2")
        nc.vector.tensor_tensor(out=t2_t[:], in0=gm_t[:], in1=am_t[:], op=ALU.mult)
        c1_t = work.tile([128, NH], F32, tag="c1")
        nc.vector.tensor_tensor(out=c1_t[:], in0=t1_t[:], in1=t2_t[:], op=ALU.add)

        def expand(coef_t, tag):
            ct_ps = ps_sm.tile([NH, 128], F32, space="PSUM", tag="sm")
            nc.tensor.transpose(out=ct_ps[:], in_=coef_t[:], identity=id_t[:])
            ct_sb = work.tile([NH, 128], F32, tag=f"{tag}T")
            nc.vector.tensor_copy(ct_sb[:], ct_ps[:])
            ce_ps = ps_mm.tile([128, C], F32, space="PSUM", tag="mm")
            nc.tensor.matmul(ce_ps[:], ct_sb[:], ehv, start=True, stop=True)
            return ce_ps

        c0e_ps = expand(c0_t, "c0")
        c1e_ps = expand(c1_t, "c1")
        att_t = work.tile([128, C], F32, tag="att")
        nc.vector.tensor_tensor(out=att_t[:], in0=c0e_ps[:], in1=vm_t[:], op=ALU.mult)
        t3_t = work.tile([128, C], F32, tag="t3")
        nc.vector.tensor_tensor(out=t3_t[:], in0=c1e_ps[:], in1=dv_t[:], op=ALU.mult)
        nc.vector.tensor_tensor(out=att_t[:], in0=att_t[:], in1=t3_t[:], op=ALU.add)

        # ---- LayerNorm + ReLU branch
        mu_t = work.tile([128, 1], F32, tag="mu")
        nc.vector.reduce_sum(out=mu_t[:], in_=h_t[:], axis=AX.X)
        nc.scalar.mul(mu_t[:], mu_t[:], 1.0 / C)
        nc.vector.tensor_scalar_sub(h_t[:], h_t[:], mu_t[:])
        sq_t = work.tile([128, C], F32, tag="sq")
        nc.vector.tensor_tensor(out=sq_t[:], in0=h_t[:], in1=h_t[:], op=ALU.mult)
        var_t = work.tile([128, 1], F32, tag="var")
        nc.vector.reduce_sum(out=var_t[:], in_=sq_t[:], axis=AX.X)
        std_t = work.tile([128, 1], F32, tag="std")
        nc.scalar.activation(std_t[:], var_t[:], AF.Sqrt, bias=eps_t[:], scale=1.0 / C)
        rstd_t = work.tile([128, 1], F32, tag="rstd")
        nc.vector.reciprocal(rstd_t[:], std_t[:])
        nc.vector.tensor_scalar_mul(h_t[:], h_t[:], rstd_t[:])
        nc.vector.tensor_tensor(out=h_t[:], in0=h_t[:], in1=lng_t, op=ALU.mult)
        nc.vector.tensor_tensor(out=h_t[:], in0=h_t[:], in1=lnb_t, op=ALU.add)
        nc.scalar.activation(h_t[:], h_t[:], AF.Relu)

        # ---- transposes to feature-major + final contractions in one PSUM
        def transpose_to_sbuf(src_t, tag):
            tiles = []
            for ctile in range(2):
                tp_ps = ps_mm.tile([128, 128], F32, space="PSUM", tag="mm")
                nc.tensor.transpose(
                    out=tp_ps[:],
                    in_=src_t[:, 128 * ctile : 128 * (ctile + 1)],
                    identity=id_t[:],
                )
                tp_sb = work.tile([128, 128], F32, tag=f"{tag}{ctile}")
                nc.vector.tensor_copy(tp_sb[:], tp_ps[:])
                tiles.append(tp_sb)
            return tiles

        att_T = transpose_to_sbuf(att_t, "attT")
        h_T = transpose_to_sbuf(h_t, "hT")

        fin_ps = ps_mm.tile([128, C], F32, space="PSUM", tag="mm")
        nc.tensor.matmul(fin_ps[:], att_T[0][:], wo_s[0][:], start=True, stop=False)
        nc.tensor.matmul(fin_ps[:], att_T[1][:], wo_s[1][:], start=False, stop=False)
        nc.tensor.matmul(fin_ps[:], h_T[0][:], p2w_s[0][:], start=False, stop=False)
        nc.tensor.matmul(fin_ps[:], h_T[1][:], p2w_s[1][:], start=False, stop=True)
        out_t = work.tile([128, C], F32, tag="outt")
        nc.vector.tensor_tensor(out=out_t[:], in0=fin_ps[:], in1=bfin_t, op=ALU.add)
        nc.sync.dma_start(out[:], out_t[:])

    # bacc legalization: splits multi-sem waits (TRN2 allows 1 wait/inst),
    # moves matmul waits onto ldweights, allocates registers
    nc.compile()
    return nc


def _get_nc():
    if "nc" not in _NC_CACHE:
        _NC_CACHE["nc"] = _build_program()
    return _NC_CACHE["nc"]


def _gauss_kernel():
    ax = np.arange(-2.0, 3.0, dtype=np.float32)
    xx, yy = np.meshgrid(ax, ax, indexing="ij")
    k = np.exp(-(xx**2 + yy**2) / (2.0 * SIGMA**2)).astype(np.float32)
    return (k / k.sum()).reshape(-1).astype(np.float32)  # (25,)


def _padrows(arr68):
    """(68, k) packed token rows -> (128, k) padded rows at 64*(bl//2)+17*(bl%2)+n."""
    k = arr68.shape[1]
    outp = np.zeros((128, k), dtype=np.float32)
    for bl in range(BPC):
        base = 64 * (bl // 2) + N * (bl % 2)
        outp[base : base + N] = arr68[bl * N : (bl + 1) * N]
    return outp


def _unpadrows(arr128):
    rows = []
    for bl in range(BPC):
        base = 64 * (bl // 2) + N * (bl % 2)
        rows.append(arr128[base : base + N])
    return np.concatenate(rows, axis=0)


def _make_in_maps(feature_map, keypoints, meta, Wq, bq, Wk, bk, Wv, bv, Wo, bo,
                  Wg, bg, P1w, P1b, ln_g, ln_b, P2w, P2b):
    f32 = lambda a: np.ascontiguousarray(np.asarray(a), dtype=np.float32)
    feature_map = f32(feature_map)
    keypoints = f32(keypoints)

    # keypoint -> feature-map integer coords.  The oracle reference runs on the
    # in-container jax (axon/neuron platform), whose f32->int32 cast is
    # round-to-nearest-even (np.rint) -- NOT C-style truncation like CPU jax.
    xs = keypoints[..., 0] * np.float32(W / ORIG_W)
    ys = keypoints[..., 1] * np.float32(H / ORIG_H)
    xi = np.clip(np.rint(xs).astype(np.int32), HALF, W - HALF - 1)  # (B, N)
    yi = np.clip(np.rint(ys).astype(np.int32), HALF, H - HALF - 1)

    g = _gauss_kernel()

    # pk blob [128, PK128]: weight slices, gate weights, meta^T, gauss, 0.5*E
    pk = np.zeros((128, PK128), dtype=np.float32)
    def put2(off, wmat):
        w = f32(wmat)
        pk[:, off : off + C] = w[:128]
        pk[:, off + C : off + 2 * C] = w[128:]
    put2(0, Wq); put2(512, Wk); put2(1024, Wv); put2(1536, Wo)
    p1 = f32(P1w)
    for k in range(4):
        pk[:, 2048 + k * C : 2048 + (k + 1) * C] = p1[128 * k : 128 * (k + 1)]
    put2(3072, P2w)
    wgf = f32(Wg)
    pk[:, 3584:3592] = wgf[:128]
    pk[:, 3592:3600] = wgf[128:]
    mt = np.tile(f32(meta), (BPC, 1)).T  # (256, 68) packed token cols
    pk[:, 3600 : 3600 + T] = mt[:128]
    pk[:, 3600 + T : 3600 + 2 * T] = mt[128:]
    pk[:, 3736:4161] = np.broadcast_to(np.tile(g, N), (128, N * PK))
    for h in range(NH):
        pk[h, 4161 + h * DK : 4161 + (h + 1) * DK] = 0.5

    # p68 blob [128, PK68]: bias/scale rows in PADDED token layout
    p68p = np.zeros((T, PK68), dtype=np.float32)
    p68p[:, 0:C] = f32(bq)
    p68p[:, C : 2 * C] = f32(bv)
    p68p[:, 2 * C : 3 * C] = f32(P1b)
    p68p[:, 3 * C : 4 * C] = f32(ln_g)
    p68p[:, 4 * C : 5 * C] = f32(ln_b)
    p68p[:, 5 * C : 6 * C] = f32(bo) + f32(P2b)
    p68p[:, 6 * C : 6 * C + NH] = f32(bg)
    p68 = _padrows(p68p)

    common = {"pk": pk, "p68": p68}

    in_maps = []
    for m in range(NCORES):
        yc = yi[BPC * m : BPC * (m + 1)].reshape(T) - HALF
        xc = xi[BPC * m : BPC * (m + 1)].reshape(T) - HALF
        fof = (yc * W + xc).reshape(1, T).astype(np.int32)
        fm_shard = np.ascontiguousarray(feature_map[BPC * m : BPC * (m + 1)]).reshape(
            BPC * C, H * W
        )
        in_maps.append({"fm": fm_shard, "foff": fof, **common})
    return in_maps


def _run(in_maps, **kwargs):
    nc = _get_nc()
    return run_bass_kernel_spmd(nc, in_maps, core_ids=list(range(NCORES)), **kwargs)


def kernel(**inputs) -> np.ndarray:
    in_maps = _make_in_maps(**inputs)
    res = _run(in_maps)
    outs = np.stack(
        [_unpadrows(res.results[i]["out"]) for i in range(NCORES)]
    )  # (8, T, C)
    return np.ascontiguousarray(outs.reshape(B, N, C))
